# revision 10
# baseline (speedup 1.0000x reference)
"""Trainium2 Bass kernel for nn_EnhancedOFTOutputLayer — v2 (bf16 big mm).

Math (per reference):
    S = 0.5*(A - A^T) per block (A = proj_R[b], 512x512, S skew-symmetric)
    Q = (I - S) @ inv(I + S + 1e-6 I)          (Cayley, orthogonal)
    filt = blockdiag(Q) @ weight               (block-row matmuls)
    y = x @ filt^T + bias

Sharding: tensor-parallel over the 8 blocks -> core b owns output rows
[512b, 512b+512).  x^T is replicated (bf16); each core computes
y_b^T = filt_b @ x^T  ([512, 8192]) with no cross-core communication.

v2 changes vs baseline:
  - big matmul in bf16 (same 1 cyc/row as fp32r, but x DMA halves to
    67 MB/core so the DMA stream never starves the PE)
  - TCH=512 moving rows per matmul (was 256): half the instruction
    count, ~10 ns/instr issue overhead amortized 2x
  - weight loaded once as a single persistent bf16 tile (no group
    pacing), Q cast to bf16 so the filt matmul is bf16 x bf16
  - 3 Newton iterations (seed X1 = I - S gives e ~ ||S||^16 ~ 8e-4)
  - startup: pa/pat first on the sync queue, eye/bias on the gpsimd
    queue in parallel; y-out DMAs on the gpsimd queue so input FIFO
    never head-of-line blocks
"""

import numpy as np
import ml_dtypes

import concourse.bass as bass
import concourse.mybir as mybir
import concourse.tile as tile
from concourse import bacc
from concourse.bass_utils import run_bass_kernel_spmd

HID = 4096
NB = 8
BS = 512  # block size
NTOK = 8192  # 4*2048
P = 128
BC = BS // P  # 4 row-chunks per 512-mat
IC = HID // P  # 32 i-chunks
TCH = 512  # token chunk (matmul moving free dim; 512 f32 = 1 psum bank)
NT = NTOK // TCH  # 16
NEWTON_ITERS = 3
F32 = mybir.dt.float32
F32R = mybir.dt.float32r
BF16 = mybir.dt.bfloat16
BF = ml_dtypes.bfloat16

_CACHE = {}


def _build():
    nc = bacc.Bacc("TRN2", target_bir_lowering=False)

    # all host-pretiled to [P, ...contiguous...] so DMAs are slab reads
    wb_d = nc.dram_tensor("wbl", [P, BC, HID], BF16, kind="ExternalInput")
    pa_d = nc.dram_tensor("pal", [P, BC, BS], BF16, kind="ExternalInput")
    pat_d = nc.dram_tensor("patl", [P, BC, BS], BF16, kind="ExternalInput")
    eye_d = nc.dram_tensor("eyel", [P, BC, BS], F32, kind="ExternalInput")
    bias_d = nc.dram_tensor("bias2d", [P, BC], F32, kind="ExternalInput")
    xt_d = nc.dram_tensor("xtl", [NT, P, IC, TCH], BF16, kind="ExternalInput")
    yt_d = nc.dram_tensor("ytl", [NT, P, BC, TCH], F32, kind="ExternalOutput")

    with tile.TileContext(nc) as tc:
        with (
            tc.tile_pool(name="persist", bufs=1) as pp,
            # one PSUM pool + shared tag for Cayley, filt AND the big matmul:
            # bank rotation then never WARs the big-mm's first psum against
            # the filt phase's last DVE drain (distinct pools would reuse
            # the same physical banks back-to-back)
            tc.tile_pool(name="psum", bufs=6, space="PSUM") as psp,
        ):
            filtT = pp.tile([P, IC, BS], BF16, tag="filtT")
            bias_sb = pp.tile([P, BC], F32, tag="bias")
            qt_sb = pp.tile([P, BC, BS], BF16, tag="qt")
            wb_sb = pp.tile([P, BC, HID], BF16, tag="wb")
            x0 = pp.tile([P, IC, TCH], BF16, tag="x0")

            with tc.tile_pool(name="cayley", bufs=1) as cp:
                a_sb = cp.tile([P, BC, BS], BF16, tag="xt", bufs=2)
                at_sb = cp.tile([P, BC, BS], BF16, tag="t1t", bufs=2)
                eye = cp.tile([P, BC, BS], F32, tag="x", bufs=2)
                # critical path first on the sync queue: pa, pat
                nc.sync.dma_start(a_sb[:], pa_d[:])
                nc.sync.dma_start(at_sb[:], pat_d[:])
                # then bulk prefetches behind them (needed later)
                nc.sync.dma_start(wb_sb[:], wb_d[:])
                nc.sync.dma_start(x0[:], xt_d[0])
                # PE warm-up: the tensor engine ramps 0.65 -> 1.2 -> 2.4 GHz
                # only after ~3us of continuous execution.  Burn the startup
                # DMA wait on zero matmuls so the first real matmul runs at
                # full clock.  memset goes FIRST on the gpsimd engine so the
                # warm-ups don't sit behind the eye/bias SWDGE transfers.
                wz = pp.tile([P, 2, BS], BF16, tag="warmz")
                nc.gpsimd.memset(wz[:], 0.0)
                # small side loads on the gpsimd queue, in parallel
                nc.gpsimd.dma_start(eye[:], eye_d[:])
                nc.gpsimd.dma_start(bias_sb[:], bias_d[:])
                for _w in range(40):
                    wps = psp.tile([P, BS], F32, tag="warm_ps", bufs=2)
                    nc.tensor.matmul(
                        wps[:], wz[:, 0, 0:P], wz[:, 1, :],
                        start=True, stop=True,
                    )

                # Drop the 1e-6*I regularizer (below fp32r noise floor).
                # X1 = 2I - D = I + S = D^T and X1^T = I - S = N^T: the
                # Newton seed aliases the constant tiles.
                s_sb = cp.tile([P, BC, BS], F32, tag="t1")
                dt_sb = cp.tile([P, BC, BS], F32R, tag="dt")  # D^T = I+S
                nc.vector.tensor_sub(s_sb[:], a_sb[:], at_sb[:])  # 2S
                nc.vector.scalar_tensor_tensor(
                    dt_sb[:], s_sb[:], 0.5, eye[:],
                    mybir.AluOpType.mult, mybir.AluOpType.add)
                nt_sb = cp.tile([P, BC, BS], F32R, tag="nt")  # N^T = I-S
                nc.vector.scalar_tensor_tensor(
                    nt_sb[:], s_sb[:], -0.5, eye[:],
                    mybir.AluOpType.mult, mybir.AluOpType.add)
                x_sb = dt_sb

                def mm512(lhsT_tile, rhs_tile, out_sb, post=None):
                    # out = lhsT.T @ rhs for 512x512 mats in [P, BC, BS] tiles
                    for c in range(BC):
                        ps = psp.tile([P, BS], F32, tag="mm_ps")
                        for k in range(BC):
                            nc.tensor.matmul(
                                ps[:],
                                lhsT_tile[:, k, c * P:(c + 1) * P],
                                rhs_tile[:, k, :],
                                start=(k == 0),
                                stop=(k == BC - 1),
                            )
                        if post is None:
                            nc.vector.tensor_copy(out_sb[:, c, :], ps[:])
                        else:
                            post(c, ps)

                # The X^T iterate of the baseline's 3-product scheme is dead
                # code (its only consumer is the next X^T); two products per
                # iteration suffice:
                #   T1t = (D@X)^T = mm(lhsT=X,   rhs=Dt)
                #   Xn  = 2X-(DX)X = mm(lhsT=T1t, rhs=X), post 2X - ps
                for it in range(NEWTON_ITERS):
                    t1t = cp.tile([P, BC, BS], F32R, tag="t1t", bufs=2)
                    mm512(x_sb, dt_sb, t1t)          # T1t = (D@X)^T
                    xn = cp.tile([P, BC, BS], F32R, tag="x", bufs=2)

                    def post_xn(c, ps, _x=x_sb, _xn=xn):
                        # Xn = 2X - (DX)X
                        nc.vector.scalar_tensor_tensor(
                            _xn[:, c, :], _x[:, c, :], 2.0, ps[:],
                            mybir.AluOpType.mult, mybir.AluOpType.subtract)

                    mm512(t1t, x_sb, None, post=post_xn)
                    x_sb = xn

                mm512(nt_sb, x_sb, qt_sb)       # Q^T = N @ X  (commute)

                # filt^T = W_b^T @ Q^T : lhsT = W_b (natural layout), bf16
                for i in range(IC):
                    ps = psp.tile([P, BS], F32, tag="mm_ps")
                    for k in range(BC):
                        nc.tensor.matmul(
                            ps[:],
                            wb_sb[:, k, i * P:(i + 1) * P],
                            qt_sb[:, k, :],
                            start=(k == 0),
                            stop=(k == BC - 1),
                        )
                    nc.vector.tensor_copy(filtT[:, i, :], ps[:])

            # big matmul: y^T[o,t] = filt @ x^T, accumulate over i
            with (
                tc.tile_pool(name="xstream", bufs=2) as xp,
                tc.tile_pool(name="ystage", bufs=2) as yp,
            ):
                nxt = None
                for t in range(NT):
                    xtt = x0 if t == 0 else nxt  # x0 prefetched during Newton
                    if t + 1 < NT:
                        # prefetch the NEXT tile before this tile's y-out
                        # dma_starts so their activation waits never
                        # head-of-line block the input stream
                        nxt = xp.tile([P, IC, TCH], BF16, tag="xtile")
                        nc.sync.dma_start(nxt[:], xt_d[t + 1])
                    ys = yp.tile([P, BC, TCH], F32, tag="ys")
                    for o in range(BC):
                        ps = psp.tile([P, TCH], F32, tag="mm_ps")
                        for i in range(IC):
                            nc.tensor.matmul(
                                ps[:],
                                filtT[:, i, o * P:(o + 1) * P],
                                xtt[:, i, :],
                                start=(i == 0),
                                stop=(i == IC - 1),
                            )
                        nc.scalar.activation(
                            ys[:, o, :], ps[:],
                            mybir.ActivationFunctionType.Identity,
                            bias=bias_sb[:, o:o + 1])
                        nc.sync.dma_start(yt_d[t, :, o, :], ys[:, o, :])

    nc.finalize()
    return nc


def kernel(weight, bias, x, proj_R, layer_idx=0, _trace=False, _tmpdir=None):
    weight = np.ascontiguousarray(np.asarray(weight, dtype=np.float32))
    bias = np.ascontiguousarray(np.asarray(bias, dtype=np.float32))
    x = np.ascontiguousarray(np.asarray(x, dtype=np.float32))
    proj_R = np.ascontiguousarray(np.asarray(proj_R, dtype=np.float32))

    if "nc" not in _CACHE:
        _CACHE["nc"] = _build()
    nc = _CACHE["nc"]

    def tile_pc(m):  # [BC*P, W] -> [P, BC, W] (partition-major tiling)
        return np.ascontiguousarray(
            m.reshape(BC, P, m.shape[1]).transpose(1, 0, 2))

    xt = x.reshape(NTOK, HID).T.astype(BF)  # [HID, NTOK] bf16
    # [NT, P, IC, TCH]: xtl[t, p, c, j] = xt[c*P + p, t*TCH + j]
    xtl = np.ascontiguousarray(
        xt.reshape(IC, P, NT, TCH).transpose(2, 1, 0, 3))
    eye = tile_pc(np.eye(BS, dtype=np.float32))
    in_maps = []
    for b in range(NB):
        a = proj_R[b]
        in_maps.append({
            "wbl": tile_pc(weight[b * BS:(b + 1) * BS, :]).astype(BF),
            "pal": tile_pc(a).astype(BF),
            "patl": tile_pc(np.ascontiguousarray(a.T)).astype(BF),
            "eyel": eye,
            "bias2d": np.ascontiguousarray(
                bias[b * BS:(b + 1) * BS].reshape(BC, P).T),
            "xtl": xtl,
        })

    res = run_bass_kernel_spmd(nc, in_maps, core_ids=list(range(NB)),
                               trace=_trace, tmpdir=_tmpdir)
    out = np.empty((NTOK, HID), dtype=np.float32)
    for b in range(NB):
        # ytl[t, p, c, j] = y^T[c*P + p, t*TCH + j]
        ytb = np.ascontiguousarray(
            res.results[b]["ytl"].transpose(2, 1, 0, 3)).reshape(BS, NTOK)
        out[:, b * BS:(b + 1) * BS] = ytb.T
    if _trace:
        _CACHE["last_exec_time_ns"] = res.exec_time_ns
        _CACHE["last_results"] = res
    return out.reshape(4, 2048, HID)


# revision 12
# speedup vs baseline: 1.0083x; 1.0083x over previous
"""Trainium2 Bass kernel for nn_EnhancedOFTOutputLayer — v2 (bf16 big mm).

Math (per reference):
    S = 0.5*(A - A^T) per block (A = proj_R[b], 512x512, S skew-symmetric)
    Q = (I - S) @ inv(I + S + 1e-6 I)          (Cayley, orthogonal)
    filt = blockdiag(Q) @ weight               (block-row matmuls)
    y = x @ filt^T + bias

Sharding: tensor-parallel over the 8 blocks -> core b owns output rows
[512b, 512b+512).  x^T is replicated (bf16); each core computes
y_b^T = filt_b @ x^T  ([512, 8192]) with no cross-core communication.

v2 changes vs baseline:
  - big matmul in bf16 (same 1 cyc/row as fp32r, but x DMA halves to
    67 MB/core so the DMA stream never starves the PE)
  - TCH=512 moving rows per matmul (was 256): half the instruction
    count, ~10 ns/instr issue overhead amortized 2x
  - weight loaded once as a single persistent bf16 tile (no group
    pacing), Q cast to bf16 so the filt matmul is bf16 x bf16
  - 3 Newton iterations (seed X1 = I - S gives e ~ ||S||^16 ~ 8e-4)
  - startup: pa/pat first on the sync queue, eye/bias on the gpsimd
    queue in parallel; y-out DMAs on the gpsimd queue so input FIFO
    never head-of-line blocks
"""

import numpy as np
import ml_dtypes

import concourse.bass as bass
import concourse.mybir as mybir
import concourse.tile as tile
from concourse import bacc
from concourse.bass_utils import run_bass_kernel_spmd

HID = 4096
NB = 8
BS = 512  # block size
NTOK = 8192  # 4*2048
P = 128
BC = BS // P  # 4 row-chunks per 512-mat
IC = HID // P  # 32 i-chunks
TCH = 512  # token chunk (matmul moving free dim; 512 f32 = 1 psum bank)
NT = NTOK // TCH  # 16
NEWTON_ITERS = 3
F32 = mybir.dt.float32
F32R = mybir.dt.float32r
BF16 = mybir.dt.bfloat16
BF = ml_dtypes.bfloat16

_CACHE = {}


def _build():
    nc = bacc.Bacc("TRN2", target_bir_lowering=False)

    # all host-pretiled to [P, ...contiguous...] so DMAs are slab reads
    wb_d = nc.dram_tensor("wbl", [P, BC, HID], BF16, kind="ExternalInput")
    pa_d = nc.dram_tensor("pal", [P, BC, BS], BF16, kind="ExternalInput")
    pat_d = nc.dram_tensor("patl", [P, BC, BS], BF16, kind="ExternalInput")
    eye_d = nc.dram_tensor("eyel", [P, BC, BS], F32, kind="ExternalInput")
    bias_d = nc.dram_tensor("bias2d", [P, BC], F32, kind="ExternalInput")
    xt_d = nc.dram_tensor("xtl", [NT, P, IC, TCH], BF16, kind="ExternalInput")
    yt_d = nc.dram_tensor("ytl", [NT, P, BC, TCH], F32, kind="ExternalOutput")

    with tile.TileContext(nc) as tc:
        with (
            tc.tile_pool(name="persist", bufs=1) as pp,
            # one PSUM pool + shared tag for Cayley, filt AND the big matmul:
            # bank rotation then never WARs the big-mm's first psum against
            # the filt phase's last DVE drain (distinct pools would reuse
            # the same physical banks back-to-back)
            tc.tile_pool(name="psum", bufs=6, space="PSUM") as psp,
        ):
            filtT = pp.tile([P, IC, BS], BF16, tag="filtT")
            bias_sb = pp.tile([P, BC], F32, tag="bias")
            qt_sb = pp.tile([P, BC, BS], BF16, tag="qt")
            wb_sb = pp.tile([P, BC, HID], BF16, tag="wb")
            x0 = pp.tile([P, IC, TCH], BF16, tag="x0")

            with tc.tile_pool(name="cayley", bufs=1) as cp:
                a_sb = cp.tile([P, BC, BS], BF16, tag="xt", bufs=2)
                at_sb = cp.tile([P, BC, BS], BF16, tag="t1t", bufs=2)
                eye = cp.tile([P, BC, BS], F32, tag="eye")
                # critical path first on the sync queue: pa/pat in
                # interleaved halves so the S = 0.5*(A - A^T) chain can
                # start on the first half while the second streams
                nc.sync.dma_start(a_sb[:, 0:2, :], pa_d[:, 0:2, :])
                nc.sync.dma_start(at_sb[:, 0:2, :], pat_d[:, 0:2, :])
                nc.sync.dma_start(a_sb[:, 2:4, :], pa_d[:, 2:4, :])
                nc.sync.dma_start(at_sb[:, 2:4, :], pat_d[:, 2:4, :])
                # then bulk prefetches behind them (needed later)
                nc.sync.dma_start(wb_sb[:], wb_d[:])
                nc.sync.dma_start(x0[:], xt_d[0])
                # small side loads on the gpsimd queue, in parallel
                nc.gpsimd.dma_start(eye[:], eye_d[:])
                nc.gpsimd.dma_start(bias_sb[:], bias_d[:])

                # PE warm-up: the tensor engine ramps 0.65 -> 1.2 -> 2.4 GHz
                # only after ~3us of continuous execution.  Burn the startup
                # DMA wait on small zero matmuls so the first real matmul
                # runs at full clock (and late ones drain fast).
                wz = pp.tile([P, 2, BS], BF16, tag="warmz")
                nc.gpsimd.memset(wz[:], 0.0)
                for _w in range(40):
                    wps = psp.tile([P, P], F32, tag="warm_ps", bufs=2)
                    nc.tensor.matmul(
                        wps[:], wz[:, 0, 0:P], wz[:, 1, 0:P],
                        start=True, stop=True,
                    )

                # Drop the 1e-6*I regularizer (below fp32r noise floor).
                # X1 = 2I - D = I + S = D^T and X1^T = I - S = N^T: the
                # Newton seed aliases the constant tiles.  Chunked so the
                # first Newton matmul (k-outer, needs only chunk 0) starts
                # right after the first pa/pat halves land.
                s_sb = cp.tile([P, BC, BS], F32, tag="t1")
                dt_sb = cp.tile([P, BC, BS], F32R, tag="dt")  # D^T = I+S
                for h in (slice(0, 2), slice(2, 4)):
                    nc.vector.tensor_sub(
                        s_sb[:, h, :], a_sb[:, h, :], at_sb[:, h, :])  # 2S
                    nc.vector.scalar_tensor_tensor(
                        dt_sb[:, h, :], s_sb[:, h, :], 0.5, eye[:, h, :],
                        mybir.AluOpType.mult, mybir.AluOpType.add)
                x_sb = dt_sb

                def mm512(lhsT_tile, rhs_tile, out_sb, post=None):
                    # out = lhsT.T @ rhs for 512x512 mats in [P, BC, BS]
                    # tiles.  k-OUTER order over 4 psum banks: the first
                    # matmul needs only chunk k=0 of the inputs, so each
                    # product starts while its operands are still being
                    # produced (DVE/DMA) and product boundaries don't stall
                    # the PE.  Chunk c's copy-out issues right after its
                    # k=BC-1 matmul.
                    pss = [psp.tile([P, BS], F32, tag="mm_ps", name=f"ps{c}")
                           for c in range(BC)]
                    for k in range(BC):
                        for c in range(BC):
                            nc.tensor.matmul(
                                pss[c][:],
                                lhsT_tile[:, k, c * P:(c + 1) * P],
                                rhs_tile[:, k, :],
                                start=(k == 0),
                                stop=(k == BC - 1),
                            )
                            if k == BC - 1:
                                if post is None:
                                    nc.vector.tensor_copy(
                                        out_sb[:, c, :], pss[c][:])
                                else:
                                    post(c, pss[c])

                # The X^T iterate of the baseline's 3-product scheme is dead
                # code (its only consumer is the next X^T); two products per
                # iteration suffice:
                #   T1t = (D@X)^T = mm(lhsT=X,   rhs=Dt)
                #   Xn  = 2X-(DX)X = mm(lhsT=T1t, rhs=X), post 2X - ps
                for it in range(NEWTON_ITERS):
                    t1t = cp.tile([P, BC, BS], F32R, tag="t1t", bufs=2)
                    mm512(x_sb, dt_sb, t1t)          # T1t = (D@X)^T
                    xn = cp.tile([P, BC, BS], F32R, tag="x", bufs=2)

                    def post_xn(c, ps, _x=x_sb, _xn=xn):
                        # Xn = 2X - (DX)X
                        nc.vector.scalar_tensor_tensor(
                            _xn[:, c, :], _x[:, c, :], 2.0, ps[:],
                            mybir.AluOpType.mult, mybir.AluOpType.subtract)

                    mm512(t1t, x_sb, None, post=post_xn)
                    x_sb = xn
                    if it == NEWTON_ITERS - 2:
                        # N^T = I-S, needed only by the final product; the
                        # STT runs on the DVE during the last iteration's
                        # matmuls instead of delaying the first product
                        nt_sb = cp.tile([P, BC, BS], F32R, tag="nt")
                        nc.vector.scalar_tensor_tensor(
                            nt_sb[:], s_sb[:], -0.5, eye[:],
                            mybir.AluOpType.mult, mybir.AluOpType.add)

                mm512(nt_sb, x_sb, qt_sb)       # Q^T = N @ X  (commute)

                # filt^T = W_b^T @ Q^T : lhsT = W_b (natural layout), bf16
                for i in range(IC):
                    ps = psp.tile([P, BS], F32, tag="mm_ps")
                    for k in range(BC):
                        nc.tensor.matmul(
                            ps[:],
                            wb_sb[:, k, i * P:(i + 1) * P],
                            qt_sb[:, k, :],
                            start=(k == 0),
                            stop=(k == BC - 1),
                        )
                    nc.vector.tensor_copy(filtT[:, i, :], ps[:])

            # big matmul: y^T[o,t] = filt @ x^T, accumulate over i
            with (
                tc.tile_pool(name="xstream", bufs=2) as xp,
                tc.tile_pool(name="ystage", bufs=2) as yp,
            ):
                nxt = None
                for t in range(NT):
                    xtt = x0 if t == 0 else nxt  # x0 prefetched during Newton
                    if t + 1 < NT:
                        # prefetch the NEXT tile before this tile's y-out
                        # dma_starts so their activation waits never
                        # head-of-line block the input stream
                        nxt = xp.tile([P, IC, TCH], BF16, tag="xtile")
                        nc.sync.dma_start(nxt[:], xt_d[t + 1])
                    ys = yp.tile([P, BC, TCH], F32, tag="ys")
                    for o in range(BC):
                        ps = psp.tile([P, TCH], F32, tag="mm_ps")
                        for i in range(IC):
                            nc.tensor.matmul(
                                ps[:],
                                filtT[:, i, o * P:(o + 1) * P],
                                xtt[:, i, :],
                                start=(i == 0),
                                stop=(i == IC - 1),
                            )
                        nc.scalar.activation(
                            ys[:, o, :], ps[:],
                            mybir.ActivationFunctionType.Identity,
                            bias=bias_sb[:, o:o + 1])
                        nc.sync.dma_start(yt_d[t, :, o, :], ys[:, o, :])

    nc.finalize()
    return nc


def kernel(weight, bias, x, proj_R, layer_idx=0, _trace=False, _tmpdir=None):
    weight = np.ascontiguousarray(np.asarray(weight, dtype=np.float32))
    bias = np.ascontiguousarray(np.asarray(bias, dtype=np.float32))
    x = np.ascontiguousarray(np.asarray(x, dtype=np.float32))
    proj_R = np.ascontiguousarray(np.asarray(proj_R, dtype=np.float32))

    if "nc" not in _CACHE:
        _CACHE["nc"] = _build()
    nc = _CACHE["nc"]

    def tile_pc(m):  # [BC*P, W] -> [P, BC, W] (partition-major tiling)
        return np.ascontiguousarray(
            m.reshape(BC, P, m.shape[1]).transpose(1, 0, 2))

    xt = x.reshape(NTOK, HID).T.astype(BF)  # [HID, NTOK] bf16
    # [NT, P, IC, TCH]: xtl[t, p, c, j] = xt[c*P + p, t*TCH + j]
    xtl = np.ascontiguousarray(
        xt.reshape(IC, P, NT, TCH).transpose(2, 1, 0, 3))
    eye = tile_pc(np.eye(BS, dtype=np.float32))
    in_maps = []
    for b in range(NB):
        a = proj_R[b]
        in_maps.append({
            "wbl": tile_pc(weight[b * BS:(b + 1) * BS, :]).astype(BF),
            "pal": tile_pc(a).astype(BF),
            "patl": tile_pc(np.ascontiguousarray(a.T)).astype(BF),
            "eyel": eye,
            "bias2d": np.ascontiguousarray(
                bias[b * BS:(b + 1) * BS].reshape(BC, P).T),
            "xtl": xtl,
        })

    res = run_bass_kernel_spmd(nc, in_maps, core_ids=list(range(NB)),
                               trace=_trace, tmpdir=_tmpdir)
    out = np.empty((NTOK, HID), dtype=np.float32)
    for b in range(NB):
        # ytl[t, p, c, j] = y^T[c*P + p, t*TCH + j]
        ytb = np.ascontiguousarray(
            res.results[b]["ytl"].transpose(2, 1, 0, 3)).reshape(BS, NTOK)
        out[:, b * BS:(b + 1) * BS] = ytb.T
    if _trace:
        _CACHE["last_exec_time_ns"] = res.exec_time_ns
        _CACHE["last_results"] = res
    return out.reshape(4, 2048, HID)


# revision 14
# speedup vs baseline: 1.0125x; 1.0041x over previous
"""Trainium2 Bass kernel for nn_EnhancedOFTOutputLayer — v2 (bf16 big mm).

Math (per reference):
    S = 0.5*(A - A^T) per block (A = proj_R[b], 512x512, S skew-symmetric)
    Q = (I - S) @ inv(I + S + 1e-6 I)          (Cayley, orthogonal)
    filt = blockdiag(Q) @ weight               (block-row matmuls)
    y = x @ filt^T + bias

Sharding: tensor-parallel over the 8 blocks -> core b owns output rows
[512b, 512b+512).  x^T is replicated (bf16); each core computes
y_b^T = filt_b @ x^T  ([512, 8192]) with no cross-core communication.

v2 changes vs baseline:
  - big matmul in bf16 (same 1 cyc/row as fp32r, but x DMA halves to
    67 MB/core so the DMA stream never starves the PE)
  - TCH=512 moving rows per matmul (was 256): half the instruction
    count, ~10 ns/instr issue overhead amortized 2x
  - weight loaded once as a single persistent bf16 tile (no group
    pacing), Q cast to bf16 so the filt matmul is bf16 x bf16
  - 3 Newton iterations (seed X1 = I - S gives e ~ ||S||^16 ~ 8e-4)
  - startup: pa/pat first on the sync queue, eye/bias on the gpsimd
    queue in parallel; y-out DMAs on the gpsimd queue so input FIFO
    never head-of-line blocks
"""

import numpy as np
import ml_dtypes

import concourse.bass as bass
import concourse.mybir as mybir
import concourse.tile as tile
from concourse import bacc
from concourse.bass_utils import run_bass_kernel_spmd

HID = 4096
NB = 8
BS = 512  # block size
NTOK = 8192  # 4*2048
P = 128
BC = BS // P  # 4 row-chunks per 512-mat
IC = HID // P  # 32 i-chunks
TCH = 512  # token chunk (matmul moving free dim; 512 f32 = 1 psum bank)
NT = NTOK // TCH  # 16
NEWTON_ITERS = 3
F32 = mybir.dt.float32
F32R = mybir.dt.float32r
BF16 = mybir.dt.bfloat16
BF = ml_dtypes.bfloat16

_CACHE = {}


def _build():
    nc = bacc.Bacc("TRN2", target_bir_lowering=False)

    # all host-pretiled to [P, ...contiguous...] so DMAs are slab reads
    wb_d = nc.dram_tensor("wbl", [P, BC, HID], BF16, kind="ExternalInput")
    pa_d = nc.dram_tensor("pal", [P, BC, BS], BF16, kind="ExternalInput")
    pat_d = nc.dram_tensor("patl", [P, BC, BS], BF16, kind="ExternalInput")
    bias_d = nc.dram_tensor("bias2d", [P, BC], F32, kind="ExternalInput")
    xt_d = nc.dram_tensor("xtl", [NT, P, IC, TCH], BF16, kind="ExternalInput")
    yt_d = nc.dram_tensor("ytl", [NT, P, BC, TCH], F32, kind="ExternalOutput")

    with tile.TileContext(nc) as tc:
        with (
            tc.tile_pool(name="persist", bufs=1) as pp,
            # one PSUM pool + shared tag for Cayley, filt AND the big matmul:
            # bank rotation then never WARs the big-mm's first psum against
            # the filt phase's last DVE drain (distinct pools would reuse
            # the same physical banks back-to-back)
            tc.tile_pool(name="psum", bufs=6, space="PSUM") as psp,
        ):
            filtT = pp.tile([P, IC, BS], BF16, tag="filtT")
            bias_sb = pp.tile([P, BC], F32, tag="bias")
            qt_sb = pp.tile([P, BC, BS], BF16, tag="qt")
            wb_sb = pp.tile([P, BC, HID], BF16, tag="wb")
            x0 = pp.tile([P, IC, TCH], BF16, tag="x0")

            with tc.tile_pool(name="cayley", bufs=1) as cp:
                a_sb = cp.tile([P, BC, BS], BF16, tag="xt", bufs=2)
                at_sb = cp.tile([P, BC, BS], BF16, tag="t1t", bufs=2)
                eye = cp.tile([P, BC, BS], F32, tag="eye")
                # critical path first on the sync queue: pa, pat
                nc.sync.dma_start(a_sb[:], pa_d[:])
                nc.sync.dma_start(at_sb[:], pat_d[:])
                # then bulk prefetches behind them (needed later)
                nc.sync.dma_start(wb_sb[:], wb_d[:])
                nc.sync.dma_start(x0[:], xt_d[0])
                nc.gpsimd.dma_start(bias_sb[:], bias_d[:])
                # identity built on-chip (ones memset + affine predicate
                # j - 128c - p == 0), off the erratic SWDGE path entirely
                nc.gpsimd.memset(eye[:], 1.0)
                nc.gpsimd.affine_select(
                    eye[:], eye[:], [[-128, BC], [1, BS]],
                    mybir.AluOpType.is_equal, 0.0,
                    base=0, channel_multiplier=-1)

                # PE warm-up: the tensor engine ramps 0.65 -> 1.2 -> 2.4 GHz
                # only after ~3us of continuous execution.  Burn the startup
                # DMA wait on small zero matmuls so the first real matmul
                # runs at full clock (and late ones drain fast).
                wz = pp.tile([P, 2, BS], BF16, tag="warmz")
                nc.gpsimd.memset(wz[:], 0.0)
                for _w in range(46):
                    wps = psp.tile([P, P], F32, tag="warm_ps", bufs=2)
                    nc.tensor.matmul(
                        wps[:], wz[:, 0, 0:P], wz[:, 1, 0:P],
                        start=True, stop=True,
                    )

                # Drop the 1e-6*I regularizer (below fp32r noise floor).
                # X1 = 2I - D = I + S = D^T and X1^T = I - S = N^T: the
                # Newton seed aliases the constant tiles.  Chunked so the
                # first Newton matmul (k-outer, needs only chunk 0) starts
                # right after the first pa/pat halves land.
                s_sb = cp.tile([P, BC, BS], F32, tag="t1")
                dt_sb = cp.tile([P, BC, BS], F32R, tag="dt")  # D^T = I+S
                for h in (slice(0, 2), slice(2, 4)):
                    nc.vector.tensor_sub(
                        s_sb[:, h, :], a_sb[:, h, :], at_sb[:, h, :])  # 2S
                    nc.vector.scalar_tensor_tensor(
                        dt_sb[:, h, :], s_sb[:, h, :], 0.5, eye[:, h, :],
                        mybir.AluOpType.mult, mybir.AluOpType.add)
                x_sb = dt_sb

                def mm512(lhsT_tile, rhs_tile, out_sb, post=None):
                    # out = lhsT.T @ rhs for 512x512 mats in [P, BC, BS]
                    # tiles.  k-OUTER order over 4 psum banks: the first
                    # matmul needs only chunk k=0 of the inputs, so each
                    # product starts while its operands are still being
                    # produced (DVE/DMA) and product boundaries don't stall
                    # the PE.  Chunk c's copy-out issues right after its
                    # k=BC-1 matmul.
                    pss = [psp.tile([P, BS], F32, tag="mm_ps", name=f"ps{c}")
                           for c in range(BC)]
                    for k in range(BC):
                        for c in range(BC):
                            nc.tensor.matmul(
                                pss[c][:],
                                lhsT_tile[:, k, c * P:(c + 1) * P],
                                rhs_tile[:, k, :],
                                start=(k == 0),
                                stop=(k == BC - 1),
                            )
                            if k == BC - 1:
                                if post is None:
                                    nc.vector.tensor_copy(
                                        out_sb[:, c, :], pss[c][:])
                                else:
                                    post(c, pss[c])

                # The X^T iterate of the baseline's 3-product scheme is dead
                # code (its only consumer is the next X^T); two products per
                # iteration suffice:
                #   T1t = (D@X)^T = mm(lhsT=X,   rhs=Dt)
                #   Xn  = 2X-(DX)X = mm(lhsT=T1t, rhs=X), post 2X - ps
                for it in range(NEWTON_ITERS):
                    t1t = cp.tile([P, BC, BS], F32R, tag="t1t", bufs=2)
                    mm512(x_sb, dt_sb, t1t)          # T1t = (D@X)^T
                    xn = cp.tile([P, BC, BS], F32R, tag="x", bufs=2)

                    def post_xn(c, ps, _x=x_sb, _xn=xn):
                        # Xn = 2X - (DX)X
                        nc.vector.scalar_tensor_tensor(
                            _xn[:, c, :], _x[:, c, :], 2.0, ps[:],
                            mybir.AluOpType.mult, mybir.AluOpType.subtract)

                    mm512(t1t, x_sb, None, post=post_xn)
                    x_sb = xn
                    if it == NEWTON_ITERS - 2:
                        # N^T = I-S, needed only by the final product; the
                        # STT runs on the DVE during the last iteration's
                        # matmuls instead of delaying the first product
                        nt_sb = cp.tile([P, BC, BS], F32R, tag="nt")
                        nc.vector.scalar_tensor_tensor(
                            nt_sb[:], s_sb[:], -0.5, eye[:],
                            mybir.AluOpType.mult, mybir.AluOpType.add)

                mm512(nt_sb, x_sb, qt_sb)       # Q^T = N @ X  (commute)

                # filt^T = W_b^T @ Q^T : lhsT = W_b (natural layout), bf16
                for i in range(IC):
                    ps = psp.tile([P, BS], F32, tag="mm_ps")
                    for k in range(BC):
                        nc.tensor.matmul(
                            ps[:],
                            wb_sb[:, k, i * P:(i + 1) * P],
                            qt_sb[:, k, :],
                            start=(k == 0),
                            stop=(k == BC - 1),
                        )
                    nc.vector.tensor_copy(filtT[:, i, :], ps[:])

            # big matmul: y^T[o,t] = filt @ x^T, accumulate over i
            with (
                tc.tile_pool(name="xstream", bufs=2) as xp,
                tc.tile_pool(name="ystage", bufs=2) as yp,
            ):
                nxt = None
                for t in range(NT):
                    xtt = x0 if t == 0 else nxt  # x0 prefetched during Newton
                    if t + 1 < NT:
                        # prefetch the NEXT tile before this tile's y-out
                        # dma_starts so their activation waits never
                        # head-of-line block the input stream
                        nxt = xp.tile([P, IC, TCH], BF16, tag="xtile")
                        nc.sync.dma_start(nxt[:], xt_d[t + 1])
                    ys = yp.tile([P, BC, TCH], F32, tag="ys")
                    for o in range(BC):
                        ps = psp.tile([P, TCH], F32, tag="mm_ps")
                        for i in range(IC):
                            nc.tensor.matmul(
                                ps[:],
                                filtT[:, i, o * P:(o + 1) * P],
                                xtt[:, i, :],
                                start=(i == 0),
                                stop=(i == IC - 1),
                            )
                        nc.scalar.activation(
                            ys[:, o, :], ps[:],
                            mybir.ActivationFunctionType.Identity,
                            bias=bias_sb[:, o:o + 1])
                        nc.sync.dma_start(yt_d[t, :, o, :], ys[:, o, :])

    nc.finalize()
    return nc


def kernel(weight, bias, x, proj_R, layer_idx=0, _trace=False, _tmpdir=None):
    weight = np.ascontiguousarray(np.asarray(weight, dtype=np.float32))
    bias = np.ascontiguousarray(np.asarray(bias, dtype=np.float32))
    x = np.ascontiguousarray(np.asarray(x, dtype=np.float32))
    proj_R = np.ascontiguousarray(np.asarray(proj_R, dtype=np.float32))

    if "nc" not in _CACHE:
        _CACHE["nc"] = _build()
    nc = _CACHE["nc"]

    def tile_pc(m):  # [BC*P, W] -> [P, BC, W] (partition-major tiling)
        return np.ascontiguousarray(
            m.reshape(BC, P, m.shape[1]).transpose(1, 0, 2))

    xt = x.reshape(NTOK, HID).T.astype(BF)  # [HID, NTOK] bf16
    # [NT, P, IC, TCH]: xtl[t, p, c, j] = xt[c*P + p, t*TCH + j]
    xtl = np.ascontiguousarray(
        xt.reshape(IC, P, NT, TCH).transpose(2, 1, 0, 3))
    eye = tile_pc(np.eye(BS, dtype=np.float32))
    in_maps = []
    for b in range(NB):
        a = proj_R[b]
        in_maps.append({
            "wbl": tile_pc(weight[b * BS:(b + 1) * BS, :]).astype(BF),
            "pal": tile_pc(a).astype(BF),
            "patl": tile_pc(np.ascontiguousarray(a.T)).astype(BF),
            "eyel": eye,
            "bias2d": np.ascontiguousarray(
                bias[b * BS:(b + 1) * BS].reshape(BC, P).T),
            "xtl": xtl,
        })

    res = run_bass_kernel_spmd(nc, in_maps, core_ids=list(range(NB)),
                               trace=_trace, tmpdir=_tmpdir)
    out = np.empty((NTOK, HID), dtype=np.float32)
    for b in range(NB):
        # ytl[t, p, c, j] = y^T[c*P + p, t*TCH + j]
        ytb = np.ascontiguousarray(
            res.results[b]["ytl"].transpose(2, 1, 0, 3)).reshape(BS, NTOK)
        out[:, b * BS:(b + 1) * BS] = ytb.T
    if _trace:
        _CACHE["last_exec_time_ns"] = res.exec_time_ns
        _CACHE["last_results"] = res
    return out.reshape(4, 2048, HID)


# revision 15
# speedup vs baseline: 1.0164x; 1.0038x over previous
"""Trainium2 Bass kernel for nn_EnhancedOFTOutputLayer — v2 (bf16 big mm).

Math (per reference):
    S = 0.5*(A - A^T) per block (A = proj_R[b], 512x512, S skew-symmetric)
    Q = (I - S) @ inv(I + S + 1e-6 I)          (Cayley, orthogonal)
    filt = blockdiag(Q) @ weight               (block-row matmuls)
    y = x @ filt^T + bias

Sharding: tensor-parallel over the 8 blocks -> core b owns output rows
[512b, 512b+512).  x^T is replicated (bf16); each core computes
y_b^T = filt_b @ x^T  ([512, 8192]) with no cross-core communication.

v2 changes vs baseline:
  - big matmul in bf16 (same 1 cyc/row as fp32r, but x DMA halves to
    67 MB/core so the DMA stream never starves the PE)
  - TCH=512 moving rows per matmul (was 256): half the instruction
    count, ~10 ns/instr issue overhead amortized 2x
  - weight loaded once as a single persistent bf16 tile (no group
    pacing), Q cast to bf16 so the filt matmul is bf16 x bf16
  - 3 Newton iterations (seed X1 = I - S gives e ~ ||S||^16 ~ 8e-4)
  - startup: pa/pat first on the sync queue, eye/bias on the gpsimd
    queue in parallel; y-out DMAs on the gpsimd queue so input FIFO
    never head-of-line blocks
"""

import numpy as np
import ml_dtypes

import concourse.bass as bass
import concourse.mybir as mybir
import concourse.tile as tile
from concourse import bacc
from concourse.bass_utils import run_bass_kernel_spmd

HID = 4096
NB = 8
BS = 512  # block size
NTOK = 8192  # 4*2048
P = 128
BC = BS // P  # 4 row-chunks per 512-mat
IC = HID // P  # 32 i-chunks
TCH = 512  # token chunk (matmul moving free dim; 512 f32 = 1 psum bank)
NT = NTOK // TCH  # 16
NEWTON_ITERS = 3
F32 = mybir.dt.float32
F32R = mybir.dt.float32r
BF16 = mybir.dt.bfloat16
BF = ml_dtypes.bfloat16

_CACHE = {}


def _build():
    nc = bacc.Bacc("TRN2", target_bir_lowering=False)

    # all host-pretiled to [P, ...contiguous...] so DMAs are slab reads
    wb_d = nc.dram_tensor("wbl", [P, BC, HID], BF16, kind="ExternalInput")
    pa_d = nc.dram_tensor("pal", [P, BC, BS], BF16, kind="ExternalInput")
    pat_d = nc.dram_tensor("patl", [P, BC, BS], BF16, kind="ExternalInput")
    bias_d = nc.dram_tensor("bias2d", [P, BC], F32, kind="ExternalInput")
    xt_d = nc.dram_tensor("xtl", [NT, P, IC, TCH], BF16, kind="ExternalInput")
    yt_d = nc.dram_tensor("ytl", [NT, P, BC, TCH], F32, kind="ExternalOutput")

    with tile.TileContext(nc) as tc:
        with (
            tc.tile_pool(name="persist", bufs=1) as pp,
            # one PSUM pool + shared tag for Cayley, filt AND the big matmul:
            # bank rotation then never WARs the big-mm's first psum against
            # the filt phase's last DVE drain (distinct pools would reuse
            # the same physical banks back-to-back)
            tc.tile_pool(name="psum", bufs=6, space="PSUM") as psp,
        ):
            filtT = pp.tile([P, IC, BS], BF16, tag="filtT")
            bias_sb = pp.tile([P, BC], F32, tag="bias")
            qt_sb = pp.tile([P, BC, BS], BF16, tag="qt")
            wb_sb = pp.tile([P, BC, HID], BF16, tag="wb")
            x0 = pp.tile([P, IC, TCH], BF16, tag="x0")

            with tc.tile_pool(name="cayley", bufs=1) as cp:
                a_sb = cp.tile([P, BC, BS], BF16, tag="xt", bufs=2)
                at_sb = cp.tile([P, BC, BS], BF16, tag="t1t", bufs=2)
                eye = cp.tile([P, BC, BS], F32, tag="eye")
                # critical path first on the sync queue: pa, pat
                nc.sync.dma_start(a_sb[:], pa_d[:])
                nc.sync.dma_start(at_sb[:], pat_d[:])
                # then bulk prefetches behind them (needed later)
                nc.sync.dma_start(wb_sb[:], wb_d[:])
                nc.sync.dma_start(x0[:], xt_d[0])
                nc.gpsimd.dma_start(bias_sb[:], bias_d[:])
                # identity built on-chip (ones memset + affine predicate
                # j - 128c - p == 0), off the erratic SWDGE path entirely
                nc.gpsimd.memset(eye[:], 1.0)
                nc.gpsimd.affine_select(
                    eye[:], eye[:], [[-128, BC], [1, BS]],
                    mybir.AluOpType.is_equal, 0.0,
                    base=0, channel_multiplier=-1)

                # PE warm-up: the tensor engine ramps 0.65 -> 1.2 -> 2.4 GHz
                # only after ~3us of continuous execution.  Burn the startup
                # DMA wait on small zero matmuls so the first real matmul
                # runs at full clock (and late ones drain fast).  memset on
                # the DVE: it idles before the S chain, while the gpsimd
                # engine is stuck in framework drains until ~9us.
                wz = pp.tile([P, 2, P], BF16, tag="warmz")
                nc.vector.memset(wz[:], 0.0)
                for _w in range(60):
                    wps = psp.tile([P, P], F32, tag="warm_ps", bufs=2)
                    nc.tensor.matmul(
                        wps[:], wz[:, 0, :], wz[:, 1, :],
                        start=True, stop=True,
                    )

                # Drop the 1e-6*I regularizer (below fp32r noise floor).
                # X1 = 2I - D = I + S = D^T and X1^T = I - S = N^T: the
                # Newton seed aliases the constant tiles.  Chunked so the
                # first Newton matmul (k-outer, needs only chunk 0) starts
                # right after the first pa/pat halves land.
                s_sb = cp.tile([P, BC, BS], F32, tag="t1")
                dt_sb = cp.tile([P, BC, BS], F32R, tag="dt")  # D^T = I+S
                for h in (slice(0, 2), slice(2, 4)):
                    nc.vector.tensor_sub(
                        s_sb[:, h, :], a_sb[:, h, :], at_sb[:, h, :])  # 2S
                    nc.vector.scalar_tensor_tensor(
                        dt_sb[:, h, :], s_sb[:, h, :], 0.5, eye[:, h, :],
                        mybir.AluOpType.mult, mybir.AluOpType.add)
                x_sb = dt_sb

                def mm512(lhsT_tile, rhs_tile, out_sb, post=None):
                    # out = lhsT.T @ rhs for 512x512 mats in [P, BC, BS]
                    # tiles.  k-OUTER order over 4 psum banks: the first
                    # matmul needs only chunk k=0 of the inputs, so each
                    # product starts while its operands are still being
                    # produced (DVE/DMA) and product boundaries don't stall
                    # the PE.  Chunk c's copy-out issues right after its
                    # k=BC-1 matmul.
                    pss = [psp.tile([P, BS], F32, tag="mm_ps", name=f"ps{c}")
                           for c in range(BC)]
                    for k in range(BC):
                        for c in range(BC):
                            nc.tensor.matmul(
                                pss[c][:],
                                lhsT_tile[:, k, c * P:(c + 1) * P],
                                rhs_tile[:, k, :],
                                start=(k == 0),
                                stop=(k == BC - 1),
                            )
                            if k == BC - 1:
                                if post is None:
                                    nc.vector.tensor_copy(
                                        out_sb[:, c, :], pss[c][:])
                                else:
                                    post(c, pss[c])

                # The X^T iterate of the baseline's 3-product scheme is dead
                # code (its only consumer is the next X^T); two products per
                # iteration suffice:
                #   T1t = (D@X)^T = mm(lhsT=X,   rhs=Dt)
                #   Xn  = 2X-(DX)X = mm(lhsT=T1t, rhs=X), post 2X - ps
                for it in range(NEWTON_ITERS):
                    t1t = cp.tile([P, BC, BS], F32R, tag="t1t", bufs=2)
                    mm512(x_sb, dt_sb, t1t)          # T1t = (D@X)^T
                    xn = cp.tile([P, BC, BS], F32R, tag="x", bufs=2)

                    def post_xn(c, ps, _x=x_sb, _xn=xn):
                        # Xn = 2X - (DX)X
                        nc.vector.scalar_tensor_tensor(
                            _xn[:, c, :], _x[:, c, :], 2.0, ps[:],
                            mybir.AluOpType.mult, mybir.AluOpType.subtract)

                    mm512(t1t, x_sb, None, post=post_xn)
                    x_sb = xn
                    if it == NEWTON_ITERS - 2:
                        # N^T = I-S, needed only by the final product; the
                        # STT runs on the DVE during the last iteration's
                        # matmuls instead of delaying the first product
                        nt_sb = cp.tile([P, BC, BS], F32R, tag="nt")
                        nc.vector.scalar_tensor_tensor(
                            nt_sb[:], s_sb[:], -0.5, eye[:],
                            mybir.AluOpType.mult, mybir.AluOpType.add)

                mm512(nt_sb, x_sb, qt_sb)       # Q^T = N @ X  (commute)

                # filt^T = W_b^T @ Q^T : lhsT = W_b (natural layout), bf16
                for i in range(IC):
                    ps = psp.tile([P, BS], F32, tag="mm_ps")
                    for k in range(BC):
                        nc.tensor.matmul(
                            ps[:],
                            wb_sb[:, k, i * P:(i + 1) * P],
                            qt_sb[:, k, :],
                            start=(k == 0),
                            stop=(k == BC - 1),
                        )
                    nc.vector.tensor_copy(filtT[:, i, :], ps[:])

            # big matmul: y^T[o,t] = filt @ x^T, accumulate over i
            with (
                tc.tile_pool(name="xstream", bufs=2) as xp,
                tc.tile_pool(name="ystage", bufs=2) as yp,
            ):
                nxt = None
                for t in range(NT):
                    xtt = x0 if t == 0 else nxt  # x0 prefetched during Newton
                    if t + 1 < NT:
                        # prefetch the NEXT tile before this tile's y-out
                        # dma_starts so their activation waits never
                        # head-of-line block the input stream
                        nxt = xp.tile([P, IC, TCH], BF16, tag="xtile")
                        nc.sync.dma_start(nxt[:], xt_d[t + 1])
                    ys = yp.tile([P, BC, TCH], F32, tag="ys")
                    for o in range(BC):
                        ps = psp.tile([P, TCH], F32, tag="mm_ps")
                        for i in range(IC):
                            nc.tensor.matmul(
                                ps[:],
                                filtT[:, i, o * P:(o + 1) * P],
                                xtt[:, i, :],
                                start=(i == 0),
                                stop=(i == IC - 1),
                            )
                        nc.scalar.activation(
                            ys[:, o, :], ps[:],
                            mybir.ActivationFunctionType.Identity,
                            bias=bias_sb[:, o:o + 1])
                        nc.sync.dma_start(yt_d[t, :, o, :], ys[:, o, :])

    nc.finalize()
    return nc


def kernel(weight, bias, x, proj_R, layer_idx=0, _trace=False, _tmpdir=None):
    weight = np.ascontiguousarray(np.asarray(weight, dtype=np.float32))
    bias = np.ascontiguousarray(np.asarray(bias, dtype=np.float32))
    x = np.ascontiguousarray(np.asarray(x, dtype=np.float32))
    proj_R = np.ascontiguousarray(np.asarray(proj_R, dtype=np.float32))

    if "nc" not in _CACHE:
        _CACHE["nc"] = _build()
    nc = _CACHE["nc"]

    def tile_pc(m):  # [BC*P, W] -> [P, BC, W] (partition-major tiling)
        return np.ascontiguousarray(
            m.reshape(BC, P, m.shape[1]).transpose(1, 0, 2))

    xt = x.reshape(NTOK, HID).T.astype(BF)  # [HID, NTOK] bf16
    # [NT, P, IC, TCH]: xtl[t, p, c, j] = xt[c*P + p, t*TCH + j]
    xtl = np.ascontiguousarray(
        xt.reshape(IC, P, NT, TCH).transpose(2, 1, 0, 3))
    eye = tile_pc(np.eye(BS, dtype=np.float32))
    in_maps = []
    for b in range(NB):
        a = proj_R[b]
        in_maps.append({
            "wbl": tile_pc(weight[b * BS:(b + 1) * BS, :]).astype(BF),
            "pal": tile_pc(a).astype(BF),
            "patl": tile_pc(np.ascontiguousarray(a.T)).astype(BF),
            "eyel": eye,
            "bias2d": np.ascontiguousarray(
                bias[b * BS:(b + 1) * BS].reshape(BC, P).T),
            "xtl": xtl,
        })

    res = run_bass_kernel_spmd(nc, in_maps, core_ids=list(range(NB)),
                               trace=_trace, tmpdir=_tmpdir)
    out = np.empty((NTOK, HID), dtype=np.float32)
    for b in range(NB):
        # ytl[t, p, c, j] = y^T[c*P + p, t*TCH + j]
        ytb = np.ascontiguousarray(
            res.results[b]["ytl"].transpose(2, 1, 0, 3)).reshape(BS, NTOK)
        out[:, b * BS:(b + 1) * BS] = ytb.T
    if _trace:
        _CACHE["last_exec_time_ns"] = res.exec_time_ns
        _CACHE["last_results"] = res
    return out.reshape(4, 2048, HID)


# revision 16
# speedup vs baseline: 1.0285x; 1.0119x over previous
"""Trainium2 Bass kernel for nn_EnhancedOFTOutputLayer — v2 (bf16 big mm).

Math (per reference):
    S = 0.5*(A - A^T) per block (A = proj_R[b], 512x512, S skew-symmetric)
    Q = (I - S) @ inv(I + S + 1e-6 I)          (Cayley, orthogonal)
    filt = blockdiag(Q) @ weight               (block-row matmuls)
    y = x @ filt^T + bias

Sharding: tensor-parallel over the 8 blocks -> core b owns output rows
[512b, 512b+512).  x^T is replicated (bf16); each core computes
y_b^T = filt_b @ x^T  ([512, 8192]) with no cross-core communication.

v2 changes vs baseline:
  - big matmul in bf16 (same 1 cyc/row as fp32r, but x DMA halves to
    67 MB/core so the DMA stream never starves the PE)
  - TCH=512 moving rows per matmul (was 256): half the instruction
    count, ~10 ns/instr issue overhead amortized 2x
  - weight loaded once as a single persistent bf16 tile (no group
    pacing), Q cast to bf16 so the filt matmul is bf16 x bf16
  - 3 Newton iterations (seed X1 = I - S gives e ~ ||S||^16 ~ 8e-4)
  - startup: pa/pat first on the sync queue, eye/bias on the gpsimd
    queue in parallel; y-out DMAs on the gpsimd queue so input FIFO
    never head-of-line blocks
"""

import numpy as np
import ml_dtypes

import concourse.bass as bass
import concourse.mybir as mybir
import concourse.tile as tile
from concourse import bacc
from concourse.bass_utils import run_bass_kernel_spmd

HID = 4096
NB = 8
BS = 512  # block size
NTOK = 8192  # 4*2048
P = 128
BC = BS // P  # 4 row-chunks per 512-mat
IC = HID // P  # 32 i-chunks
TCH = 512  # token chunk (matmul moving free dim; 512 f32 = 1 psum bank)
NT = NTOK // TCH  # 16
NEWTON_ITERS = 2
SEEDC = 0.831  # scaled seed X1 = c*(I+S): |err| <= max(|1-c|, |1-c(1+|S|^2)|) ~ 0.17
F32 = mybir.dt.float32
F32R = mybir.dt.float32r
BF16 = mybir.dt.bfloat16
BF = ml_dtypes.bfloat16

_CACHE = {}


def _build():
    nc = bacc.Bacc("TRN2", target_bir_lowering=False)

    # all host-pretiled to [P, ...contiguous...] so DMAs are slab reads
    wb_d = nc.dram_tensor("wbl", [P, BC, HID], BF16, kind="ExternalInput")
    pa_d = nc.dram_tensor("pal", [P, BC, BS], BF16, kind="ExternalInput")
    pat_d = nc.dram_tensor("patl", [P, BC, BS], BF16, kind="ExternalInput")
    bias_d = nc.dram_tensor("bias2d", [P, BC], F32, kind="ExternalInput")
    xt_d = nc.dram_tensor("xtl", [NT, P, IC, TCH], BF16, kind="ExternalInput")
    yt_d = nc.dram_tensor("ytl", [NT, P, BC, TCH], F32, kind="ExternalOutput")

    with tile.TileContext(nc) as tc:
        with (
            tc.tile_pool(name="persist", bufs=1) as pp,
            # one PSUM pool + shared tag for Cayley, filt AND the big matmul:
            # bank rotation then never WARs the big-mm's first psum against
            # the filt phase's last DVE drain (distinct pools would reuse
            # the same physical banks back-to-back)
            tc.tile_pool(name="psum", bufs=6, space="PSUM") as psp,
        ):
            filtT = pp.tile([P, IC, BS], BF16, tag="filtT")
            bias_sb = pp.tile([P, BC], F32, tag="bias")
            qt_sb = pp.tile([P, BC, BS], BF16, tag="qt")
            wb_sb = pp.tile([P, BC, HID], BF16, tag="wb")
            x0 = pp.tile([P, IC, TCH], BF16, tag="x0")

            with tc.tile_pool(name="cayley", bufs=1) as cp:
                a_sb = cp.tile([P, BC, BS], BF16, tag="xt", bufs=2)
                at_sb = cp.tile([P, BC, BS], BF16, tag="t1t", bufs=2)
                eye = cp.tile([P, BC, BS], F32, tag="eye")
                # critical path first on the sync queue: pa, pat
                nc.sync.dma_start(a_sb[:], pa_d[:])
                nc.sync.dma_start(at_sb[:], pat_d[:])
                # then bulk prefetches behind them (needed later)
                nc.sync.dma_start(wb_sb[:], wb_d[:])
                nc.sync.dma_start(x0[:], xt_d[0])
                nc.gpsimd.dma_start(bias_sb[:], bias_d[:])
                # identity built on-chip (ones memset + affine predicate
                # j - 128c - p == 0), off the erratic SWDGE path entirely
                nc.gpsimd.memset(eye[:], 1.0)
                nc.gpsimd.affine_select(
                    eye[:], eye[:], [[-128, BC], [1, BS]],
                    mybir.AluOpType.is_equal, 0.0,
                    base=0, channel_multiplier=-1)

                # PE warm-up: the tensor engine ramps 0.65 -> 1.2 -> 2.4 GHz
                # only after ~3us of continuous execution.  Burn the startup
                # DMA wait on small zero matmuls so the first real matmul
                # runs at full clock (and late ones drain fast).  memset on
                # the DVE: it idles before the S chain, while the gpsimd
                # engine is stuck in framework drains until ~9us.
                wz = pp.tile([P, 2, P], BF16, tag="warmz")
                nc.vector.memset(wz[:], 0.0)
                for _w in range(72):
                    wps = psp.tile([P, P], F32, tag="warm_ps", bufs=2)
                    nc.tensor.matmul(
                        wps[:], wz[:, 0, :], wz[:, 1, :],
                        start=True, stop=True,
                    )

                # Drop the 1e-6*I regularizer (below fp32r noise floor).
                # X1 = 2I - D = I + S = D^T and X1^T = I - S = N^T: the
                # Newton seed aliases the constant tiles.  Chunked so the
                # first Newton matmul (k-outer, needs only chunk 0) starts
                # right after the first pa/pat halves land.
                s_sb = cp.tile([P, BC, BS], F32, tag="t1")
                dt_sb = cp.tile([P, BC, BS], F32R, tag="dt")  # D^T = I+S
                for h in (slice(0, 2), slice(2, 4)):
                    nc.vector.tensor_sub(
                        s_sb[:, h, :], a_sb[:, h, :], at_sb[:, h, :])  # 2S
                    nc.vector.scalar_tensor_tensor(
                        dt_sb[:, h, :], s_sb[:, h, :], 0.5, eye[:, h, :],
                        mybir.AluOpType.mult, mybir.AluOpType.add)
                # scaled Newton seed X1 = c*D^T on the (idle) Scalar
                # engine: error 0.41 -> ~0.17, so 2 iterations reach the
                # same ~8e-4 floor as 3 unscaled ones
                x1_sb = cp.tile([P, BC, BS], F32R, tag="x", bufs=2)
                for h in (slice(0, 2), slice(2, 4)):
                    nc.scalar.activation(
                        x1_sb[:, h, :], dt_sb[:, h, :],
                        mybir.ActivationFunctionType.Identity, scale=SEEDC)
                x_sb = x1_sb

                def mm512(lhsT_tile, rhs_tile, out_sb, post=None):
                    # out = lhsT.T @ rhs for 512x512 mats in [P, BC, BS]
                    # tiles.  k-OUTER order over 4 psum banks: the first
                    # matmul needs only chunk k=0 of the inputs, so each
                    # product starts while its operands are still being
                    # produced (DVE/DMA) and product boundaries don't stall
                    # the PE.  Chunk c's copy-out issues right after its
                    # k=BC-1 matmul.
                    pss = [psp.tile([P, BS], F32, tag="mm_ps", name=f"ps{c}")
                           for c in range(BC)]
                    for k in range(BC):
                        for c in range(BC):
                            nc.tensor.matmul(
                                pss[c][:],
                                lhsT_tile[:, k, c * P:(c + 1) * P],
                                rhs_tile[:, k, :],
                                start=(k == 0),
                                stop=(k == BC - 1),
                            )
                            if k == BC - 1:
                                if post is None:
                                    nc.vector.tensor_copy(
                                        out_sb[:, c, :], pss[c][:])
                                else:
                                    post(c, pss[c])

                # The X^T iterate of the baseline's 3-product scheme is dead
                # code (its only consumer is the next X^T); two products per
                # iteration suffice:
                #   T1t = (D@X)^T = mm(lhsT=X,   rhs=Dt)
                #   Xn  = 2X-(DX)X = mm(lhsT=T1t, rhs=X), post 2X - ps
                for it in range(NEWTON_ITERS):
                    t1t = cp.tile([P, BC, BS], F32R, tag="t1t", bufs=2)
                    mm512(x_sb, dt_sb, t1t)          # T1t = (D@X)^T
                    xn = cp.tile([P, BC, BS], F32R, tag="x", bufs=2)

                    def post_xn(c, ps, _x=x_sb, _xn=xn):
                        # Xn = 2X - (DX)X
                        nc.vector.scalar_tensor_tensor(
                            _xn[:, c, :], _x[:, c, :], 2.0, ps[:],
                            mybir.AluOpType.mult, mybir.AluOpType.subtract)

                    mm512(t1t, x_sb, None, post=post_xn)
                    x_sb = xn
                    if it == NEWTON_ITERS - 2:
                        # N^T = I-S, needed only by the final product; the
                        # STT runs on the DVE during the last iteration's
                        # matmuls instead of delaying the first product
                        nt_sb = cp.tile([P, BC, BS], F32R, tag="nt")
                        nc.vector.scalar_tensor_tensor(
                            nt_sb[:], s_sb[:], -0.5, eye[:],
                            mybir.AluOpType.mult, mybir.AluOpType.add)

                mm512(nt_sb, x_sb, qt_sb)       # Q^T = N @ X  (commute)

                # filt^T = W_b^T @ Q^T : lhsT = W_b (natural layout), bf16
                for i in range(IC):
                    ps = psp.tile([P, BS], F32, tag="mm_ps")
                    for k in range(BC):
                        nc.tensor.matmul(
                            ps[:],
                            wb_sb[:, k, i * P:(i + 1) * P],
                            qt_sb[:, k, :],
                            start=(k == 0),
                            stop=(k == BC - 1),
                        )
                    nc.vector.tensor_copy(filtT[:, i, :], ps[:])

            # big matmul: y^T[o,t] = filt @ x^T, accumulate over i
            with (
                tc.tile_pool(name="xstream", bufs=2) as xp,
                tc.tile_pool(name="ystage", bufs=2) as yp,
            ):
                nxt = None
                for t in range(NT):
                    xtt = x0 if t == 0 else nxt  # x0 prefetched during Newton
                    if t + 1 < NT:
                        # prefetch the NEXT tile before this tile's y-out
                        # dma_starts so their activation waits never
                        # head-of-line block the input stream
                        nxt = xp.tile([P, IC, TCH], BF16, tag="xtile")
                        nc.sync.dma_start(nxt[:], xt_d[t + 1])
                    ys = yp.tile([P, BC, TCH], F32, tag="ys")
                    for o in range(BC):
                        ps = psp.tile([P, TCH], F32, tag="mm_ps")
                        for i in range(IC):
                            nc.tensor.matmul(
                                ps[:],
                                filtT[:, i, o * P:(o + 1) * P],
                                xtt[:, i, :],
                                start=(i == 0),
                                stop=(i == IC - 1),
                            )
                        nc.scalar.activation(
                            ys[:, o, :], ps[:],
                            mybir.ActivationFunctionType.Identity,
                            bias=bias_sb[:, o:o + 1])
                        nc.sync.dma_start(yt_d[t, :, o, :], ys[:, o, :])

    nc.finalize()
    return nc


def kernel(weight, bias, x, proj_R, layer_idx=0, _trace=False, _tmpdir=None):
    weight = np.ascontiguousarray(np.asarray(weight, dtype=np.float32))
    bias = np.ascontiguousarray(np.asarray(bias, dtype=np.float32))
    x = np.ascontiguousarray(np.asarray(x, dtype=np.float32))
    proj_R = np.ascontiguousarray(np.asarray(proj_R, dtype=np.float32))

    if "nc" not in _CACHE:
        _CACHE["nc"] = _build()
    nc = _CACHE["nc"]

    def tile_pc(m):  # [BC*P, W] -> [P, BC, W] (partition-major tiling)
        return np.ascontiguousarray(
            m.reshape(BC, P, m.shape[1]).transpose(1, 0, 2))

    xt = x.reshape(NTOK, HID).T.astype(BF)  # [HID, NTOK] bf16
    # [NT, P, IC, TCH]: xtl[t, p, c, j] = xt[c*P + p, t*TCH + j]
    xtl = np.ascontiguousarray(
        xt.reshape(IC, P, NT, TCH).transpose(2, 1, 0, 3))
    eye = tile_pc(np.eye(BS, dtype=np.float32))
    in_maps = []
    for b in range(NB):
        a = proj_R[b]
        in_maps.append({
            "wbl": tile_pc(weight[b * BS:(b + 1) * BS, :]).astype(BF),
            "pal": tile_pc(a).astype(BF),
            "patl": tile_pc(np.ascontiguousarray(a.T)).astype(BF),
            "eyel": eye,
            "bias2d": np.ascontiguousarray(
                bias[b * BS:(b + 1) * BS].reshape(BC, P).T),
            "xtl": xtl,
        })

    res = run_bass_kernel_spmd(nc, in_maps, core_ids=list(range(NB)),
                               trace=_trace, tmpdir=_tmpdir)
    out = np.empty((NTOK, HID), dtype=np.float32)
    for b in range(NB):
        # ytl[t, p, c, j] = y^T[c*P + p, t*TCH + j]
        ytb = np.ascontiguousarray(
            res.results[b]["ytl"].transpose(2, 1, 0, 3)).reshape(BS, NTOK)
        out[:, b * BS:(b + 1) * BS] = ytb.T
    if _trace:
        _CACHE["last_exec_time_ns"] = res.exec_time_ns
        _CACHE["last_results"] = res
    return out.reshape(4, 2048, HID)
